# revision 11
# baseline (speedup 1.0000x reference)
"""Trainium2 Bass kernel for nn_Blur: per-sample 3D PSF blur (grouped conv3d).

Strategy
--------
The PSF  K[z,i,j] = (1 - exp(-alpha * ax[z] * lat[i,j])) / S  with
lat[i,j] = g[i]*g[j]/(2*pi*bxy^2) is, by Taylor expansion of 1-exp(-t),
an *exact* (to fp32) rank-4 CP tensor:

    K[z,i,j] = sum_m A[z,m] * U[i,m] * U[j,m],   m = 1..4
    A[z,m] = (-1)^(m+1) c[z]^m / m! / S,  U[i,m] = g[i]^m,
    c[z] = alpha*ax[z]/(2*pi*bxy^2)

so the 3D conv factorizes into 1D convs: y-conv, x-conv, then a z-conv
fused with the rank summation.  All three stages are PE matmuls (bf16
operands, fp32 PSUM):

  SA  input arrives via direct DRAM->SBUF XBAR-transpose DMAs (bf16,
      converted host-side): TT[y, (t,g,z,x)], two overlapping y-windows
  SB  y-conv: data-stationary matmul, moving = Toeplitz(U_m on y),
      one merged psum->SBUF copy per z-plane
  SC  x-conv: Toeplitz(U_m on x)-stationary matmul, y-halved into two
      output tiles so the SD transposes can start at half-SC
  SD  XBAR transpose to put (m,z) on partitions, 4 y-chunk tiles,
      serialized on the sync ring (two concurrent SBUF->SBUF XBAR
      transposes on the two HWDGE rings corrupt each other on HW),
      pipelined with SC and SE
  SE  z-conv + rank sum: [128 -> 32] banded stationary, consumes SD
      y-chunk tiles as they land

Copies rotate across Activation/DVE (scheduler-assigned) and GpSimd.
Sharding: 8 cores = 4 samples x 2 x-halves (halo 7 in x, handled by
host-side padding).  No cross-core communication.
"""

import math
import sys

import numpy as np

for p in ("/opt/trn_rl_repo", "/root/.axon_site/_ro/trn_rl_repo"):
    if p not in sys.path:
        sys.path.append(p)

# geometry (hardcoded for this problem)
B = 4
Z, X, Y = 32, 192, 192
KZ, KT = 9, 15          # z taps; x/y taps
XH = X // 2             # 96 output x per core
XIN = XH + KT - 1       # 110 input x rows per core
YIN = Y + KT - 1        # 206 padded y
R = 4                   # CP rank (Taylor order)
NCORES = 8

_CACHE = {}


def _taylor_factors(bet_xy, bet_z, alpha):
    """Per-sample CP factors (A[9,R], U[15,R]) of the globally-normalized PSF."""
    zd = np.abs(np.arange(KZ) - KZ // 2).astype(np.float64)
    xd = np.abs(np.arange(KT) - KT // 2).astype(np.float64)
    dp = xd[:, None] ** 2 + xd[None, :] ** 2
    S = 0.0
    for b in range(B):
        bxy, bz, al = float(bet_xy[b]), float(bet_z[b]), float(alpha[b])
        lat = np.exp(-dp / (2 * bxy**2)) / (2 * np.pi * bxy**2)
        ax = np.exp(-zd**2 / (2 * bz**2)) / (np.sqrt(2 * np.pi) * bz)
        S += (1.0 - np.exp(-al * lat[None] * ax[:, None, None])).sum()
    facs = []
    for b in range(B):
        bxy, bz, al = float(bet_xy[b]), float(bet_z[b]), float(alpha[b])
        g = np.exp(-xd**2 / (2 * bxy**2))
        ax = np.exp(-zd**2 / (2 * bz**2)) / (np.sqrt(2 * np.pi) * bz)
        c = al * ax / (2 * np.pi * bxy**2)
        A = np.stack(
            [(-1) ** (m + 1) * c**m / math.factorial(m) / S for m in range(1, R + 1)], 1
        )
        U = np.stack([g**m for m in range(1, R + 1)], 1)
        facs.append((A.astype(np.float32), U.astype(np.float32)))
    return facs


def _build_mats(A, U):
    """Device weight matrices for one sample (built fp32, shipped bf16).

    ty0/ty1 hold the 4 ranks' y-Toeplitz blocks (96 outputs each) side by
    side so one matmul per (z, y-window) computes all ranks, and the two
    windows' psum blocks merge into a single copy.
    """
    ty0 = np.zeros((128, R * 96), np.float32)
    ty1 = np.zeros((128, R * 96), np.float32)
    tx = np.zeros((R, XIN, XH), np.float32)
    zm = np.zeros((128, Z), np.float32)
    for m in range(R):
        for yp in range(128):
            for yo in range(96):
                j = yp - yo          # window y_in [0:128] -> y_out 0..95
                if 0 <= j < KT:
                    ty0[yp, m * 96 + yo] = U[j, m]
                j = yp - yo - 18     # window y_in [78:206] -> y_out 96..191
                if 0 <= j < KT:
                    ty1[yp, m * 96 + yo] = U[j, m]
        for i in range(XIN):
            for o in range(max(0, i - KT + 1), min(XH, i + 1)):
                tx[m, i, o] = U[i - o, m]
        for zi in range(Z):
            for zo in range(max(0, zi - 4), min(Z, zi + 5)):
                zm[m * Z + zi, zo] = A[zi - zo + 4, m]
    return ty0, ty1, tx, zm


def _build_program(reps=1, upto=None):
    """upto: truncate after a stage ("A".."D") for stage-delta profiling;
    the last intermediate is DMA'd to out_d to keep it live through DCE."""
    import concourse.mybir as mybir
    import concourse.tile as tile
    from concourse import bacc

    F32, BF16 = mybir.dt.float32, mybir.dt.bfloat16

    nc = bacc.Bacc("TRN2", target_bir_lowering=False, debug=False, num_devices=NCORES)

    xin_d = nc.dram_tensor("xin", [Z, XIN, YIN], BF16, kind="ExternalInput")
    ty0_d = nc.dram_tensor("ty0", [128, R * 96], BF16, kind="ExternalInput")
    ty1_d = nc.dram_tensor("ty1", [128, R * 96], BF16, kind="ExternalInput")
    tx_d = nc.dram_tensor("tx", [R, XIN, XH], BF16, kind="ExternalInput")
    zm_d = nc.dram_tensor("zm", [128, Z], BF16, kind="ExternalInput")
    out_d = nc.dram_tensor("out", [Z, XH, Y], F32, kind="ExternalOutput")

    with tile.TileContext(nc) as tc:
        hwq = [nc.sync, nc.scalar]
        ncopy = [0]

        def copy(out, in_):
            # GPSIMD cannot read PSUM, so psum-evicting copies can only go
            # to Activation/DVE; let the scheduler balance those two.
            ncopy[0] += 1
            nc.any.tensor_copy(out=out, in_=in_)

        with (
            tc.tile_pool(name="consts", bufs=1) as consts,
            tc.tile_pool(name="persist", bufs=1) as persist,
            tc.tile_pool(name="psum", bufs=4, space="PSUM") as psp,
        ):
            # consts on the (otherwise idle) gpsimd queue, keeping the two
            # HWDGE rings free for the input transposes
            ty0 = consts.tile([128, R * 96], BF16)
            nc.gpsimd.dma_start(out=ty0[:], in_=ty0_d[:])
            ty1 = consts.tile([128, R * 96], BF16)
            nc.gpsimd.dma_start(out=ty1[:], in_=ty1_d[:])
            tx = [consts.tile([XIN, XH], BF16, name=f"tx_{m}") for m in range(R)]
            for m in range(R):
                nc.gpsimd.dma_start(out=tx[m][:], in_=tx_d[m])
            zmt = consts.tile([128, Z], BF16)
            nc.gpsimd.dma_start(out=zmt[:], in_=zm_d[:])

            # reps>1 (timing only): repeat the whole pipeline sequentially in
            # one NEFF; same-tag persistent tiles serialize the reps.
            for _rep in range(reps):
                # TT free layout: (t, g, z8, x110) -> block = t*4+g is one
                # contiguous 880-col chunk, written by one XBAR transpose.
                TT = persist.tile([128, 8 * 880], BF16, tag="tt")
                W = persist.tile([XIN, Z * R * Y], BF16, tag="w")
                # Xt/Wz split into per-y-chunk tiles so the SD/SE pipeline
                # gets exact tile-level dependencies.
                Xh = [
                    persist.tile([XH, 96 * R * Z], BF16, tag=f"xt{h}", name=f"xh{h}")
                    for h in range(2)
                ]
                Wzk = [
                    persist.tile([128, 48 * XH], BF16, tag=f"wz{k}", name=f"wzk{k}")
                    for k in range(4)
                ]
                Out = persist.tile([128, 24 * Y], F32, tag="outt")
                Wv = W[:].rearrange("p (z m y) -> p z m y", z=Z, m=R, y=Y)
                Wb = W[:].rearrange("p (z m b y) -> p z m b y", z=Z, m=R, b=2)
                Wc = W[:].rearrange("p (z m y) -> p m z y", z=Z, m=R, y=Y)
                Xv = [
                    Xh[h][:].rearrange("p (y m z) -> p y m z", y=96, m=R, z=Z)
                    for h in range(2)
                ]
                Wzv = [Wzk[k][:].rearrange("p (y x) -> p y x", y=48) for k in range(4)]
                OutV = Out[:].rearrange("p (x y) -> p x y", x=24)

                # SA: 4 direct DRAM->SBUF transpose DMAs (2 y-windows x 2
                # z-groups of 16).  ALL transposes in this program go on the
                # sync ring: two concurrent XBAR transposes (any source)
                # corrupt each other on HW.  SB group g needs 2 blocks.
                for g in range(2):
                    for t in range(2):
                        src = xin_d[16 * g : 16 * (g + 1), :, 78 * t : 78 * t + 128]
                        nc.sync.dma_start_transpose(
                            TT[:, (t * 2 + g) * 1760 : (t * 2 + g + 1) * 1760],
                            src.rearrange("z x y -> (z x) y"),
                        )

                if upto == "A":
                    nc.gpsimd.dma_start(
                        out=out_d[:].rearrange("z x y -> z (x y)")[:, 0:7040],
                        in_=TT[0:Z, :],
                    )
                    continue

                # SB: y-conv, all ranks per z-plane in 2 matmuls + 1 copy
                def tts(zp, t):
                    s = (t * 2 + zp // 16) * 1760 + (zp % 16) * 110
                    return TT[:, s : s + 110]

                for zp in range(Z):
                    ps = psp.tile([128, 1024], F32, tag="ps")
                    nc.tensor.matmul(ps[:XIN, 0:384], tts(zp, 0), ty0[:])
                    nc.tensor.matmul(ps[:XIN, 512:896], tts(zp, 1), ty1[:])
                    src = (
                        ps[:XIN]
                        .rearrange("p (b q) -> p b q", b=2)[:, :, 0:384]
                        .rearrange("p b (m y) -> p m b y", m=R)
                    )
                    copy(out=Wb[:, zp], in_=src)

                if upto == "B":
                    nc.gpsimd.dma_start(
                        out=out_d[:].rearrange("z x y -> z (x y)"),
                        in_=W[0:Z, 0 : XH * Y],
                    )
                    continue

                # SC: x-conv, y-halved: per (h, m, z-octet) 2 z-quad matmuls
                # (N=384) + 1 copy.  Finishing half h unblocks SD chunks 2h(+1).
                for h in range(2):
                    for m in range(R):
                        for zq in range(4):
                            z0 = 8 * zq
                            ps = psp.tile([128, 1024], F32, tag="ps")
                            for i in range(2):
                                nc.tensor.matmul(
                                    ps[:XH, 512 * i : 512 * i + 384],
                                    tx[m][:],
                                    Wc[
                                        :,
                                        m,
                                        z0 + 4 * i : z0 + 4 * i + 4,
                                        96 * h : 96 * h + 96,
                                    ],
                                )
                            src = (
                                ps[:XH]
                                .rearrange("p (b q) -> p b q", b=2)[:, :, 0:384]
                                .rearrange("p b (z y) -> p b z y", z=4)
                            )
                            dst = Xv[h][:, :, m, z0 : z0 + 8].rearrange(
                                "p y (b z) -> p b z y", b=2
                            )
                            copy(out=dst, in_=src)

                if upto == "C":
                    nc.gpsimd.dma_start(
                        out=out_d[:].rearrange("z x y -> z (x y)")[:, 0 : 96 * R * Z],
                        in_=Xh[0][0:Z, :],
                    )
                    continue

                # SD: (m,z)-transpose, 4 y-chunk tiles, serialized on the sync
                # ring (concurrent SBUF->SBUF XBAR transposes corrupt on HW).
                for k in range(4):
                    nc.sync.dma_start_transpose(
                        Wzv[k][:, :, :],
                        Xh[k // 2][:, (k % 2) * 48 * 128 : (k % 2 + 1) * 48 * 128],
                    )

                if upto == "D":
                    nc.gpsimd.dma_start(
                        out=out_d[:].rearrange("z x y -> z (x y)")[:, 0 : 48 * XH],
                        in_=Wzk[0][0:Z, :],
                    )
                    continue

                # SE: z-conv + rank sum per (y-chunk, x8-grain); consumes SD
                # chunk tiles as they land.
                for yb in range(4):
                    for xp in range(6):
                        ps = psp.tile([128, 1024], F32, tag="ps")
                        for i in range(2):
                            nc.tensor.matmul(
                                ps[:Z, 512 * i : 512 * i + 384],
                                zmt[:],
                                Wzv[yb][:, :, 8 * (2 * xp + i) : 8 * (2 * xp + i) + 8],
                            )
                        xg0, xl0 = (16 * xp) // 24, (16 * xp) % 24
                        xg1 = (16 * xp + 8) // 24
                        if xg0 == xg1:
                            # both x8-grains in one xg block: single copy
                            copy(
                                out=OutV[
                                    xg0 * Z : (xg0 + 1) * Z,
                                    xl0 : xl0 + 16,
                                    48 * yb : 48 * (yb + 1),
                                ].rearrange("p (i x) y -> p i x y", i=2),
                                in_=ps[:Z]
                                .rearrange("p (i q) -> p i q", i=2)[:, :, 0:384]
                                .rearrange("p i (y x) -> p i x y", y=48),
                            )
                        else:
                            for i in range(2):
                                xg, xl = (16 * xp + 8 * i) // 24, (16 * xp + 8 * i) % 24
                                copy(
                                    out=OutV[
                                        xg * Z : (xg + 1) * Z,
                                        xl : xl + 8,
                                        48 * yb : 48 * (yb + 1),
                                    ],
                                    in_=ps[:Z, 512 * i : 512 * i + 384].rearrange(
                                        "p (y x) -> p x y", y=48
                                    ),
                                )

                # out DRAM [z, x, y] <- Out [(xg z), (xl y)]; one DMA per xg,
                # on the scalar ring so they never queue behind transposes.
                for xg in range(4):
                    nc.scalar.dma_start(
                        out=out_d[:, xg * 24 : (xg + 1) * 24, :],
                        in_=Out[xg * Z : (xg + 1) * Z, :],
                    )

    nc.compile()
    return nc


def _make_in_maps(x, bet_xy, bet_z, alpha):
    import ml_dtypes

    bf16 = ml_dtypes.bfloat16
    facs = _taylor_factors(np.asarray(bet_xy), np.asarray(bet_z), np.asarray(alpha))
    in_maps = []
    for c in range(NCORES):
        b, xh = c // 2, c % 2
        A, U = facs[b]
        ty0, ty1, tx, zm = _build_mats(A, U)
        xpad = np.zeros((Z, XIN, YIN), np.float32)
        x0 = XH * xh - 7
        lo, hi = max(0, x0), min(X, x0 + XIN)
        xpad[:, lo - x0 : hi - x0, 7 : 7 + Y] = x[b, 0, :, lo:hi, :]
        in_maps.append(
            {
                "xin": xpad.astype(bf16),
                "ty0": ty0.astype(bf16),
                "ty1": ty1.astype(bf16),
                "tx": tx.astype(bf16),
                "zm": zm.astype(bf16),
            }
        )
    return in_maps


def kernel(x, bet_xy, bet_z, alpha):
    from concourse.bass_utils import run_bass_kernel_spmd

    x = np.asarray(x, dtype=np.float32)

    if "nc" not in _CACHE:
        _CACHE["nc"] = _build_program()
    nc = _CACHE["nc"]

    in_maps = _make_in_maps(x, bet_xy, bet_z, alpha)
    res = run_bass_kernel_spmd(nc, in_maps, list(range(NCORES))).results

    out = np.empty((B, 1, Z, X, Y), np.float32)
    for c in range(NCORES):
        b, xh = c // 2, c % 2
        out[b, 0, :, XH * xh : XH * (xh + 1), :] = res[c]["out"]
    return out


# revision 14
# speedup vs baseline: 1.9656x; 1.9656x over previous
"""Trainium2 Bass kernel for nn_Blur: per-sample 3D PSF blur (grouped conv3d).

Strategy
--------
The PSF  K[z,i,j] = (1 - exp(-alpha * ax[z] * lat[i,j])) / S  with
lat[i,j] = g[i]*g[j]/(2*pi*bxy^2) is, by Taylor expansion of 1-exp(-t),
an *exact* (to fp32) rank-4 CP tensor:

    K[z,i,j] = sum_m A[z,m] * U[i,m] * U[j,m],   m = 1..4
    A[z,m] = (-1)^(m+1) c[z]^m / m! / S,  U[i,m] = g[i]^m,
    c[z] = alpha*ax[z]/(2*pi*bxy^2)

so the 3D conv factorizes into 1D convs: y-conv, x-conv, then a z-conv
fused with the rank summation.  All three stages are PE matmuls (bf16
operands, fp32 PSUM):

  SA  input arrives via direct DRAM->SBUF XBAR-transpose DMAs (bf16,
      converted host-side): TT[y, (t,g,z,x)], two overlapping y-windows
  SB  y-conv: data-stationary matmul, moving = Toeplitz(U_m on y),
      one merged psum->SBUF copy per z-plane
  SC  x-conv: Toeplitz(U_m on x)-stationary matmul, y-halved into two
      output tiles so the SD transposes can start at half-SC
  SD  XBAR transpose to put (m,z) on partitions, 4 y-chunk tiles,
      serialized on the sync ring (two concurrent SBUF->SBUF XBAR
      transposes on the two HWDGE rings corrupt each other on HW),
      pipelined with SC and SE
  SE  z-conv + rank sum: [128 -> 32] banded stationary, consumes SD
      y-chunk tiles as they land

Copies rotate across Activation/DVE (scheduler-assigned) and GpSimd.
Sharding: 8 cores = 4 samples x 2 x-halves (halo 7 in x, handled by
host-side padding).  No cross-core communication.
"""

import math
import sys

import numpy as np

for p in ("/opt/trn_rl_repo", "/root/.axon_site/_ro/trn_rl_repo"):
    if p not in sys.path:
        sys.path.append(p)

# geometry (hardcoded for this problem)
B = 4
Z, X, Y = 32, 192, 192
KZ, KT = 9, 15          # z taps; x/y taps
XH = X // 2             # 96 output x per core
XIN = XH + KT - 1       # 110 input x rows per core
YIN = Y + KT - 1        # 206 padded y
R = 4                   # CP rank (Taylor order)
NCORES = 8

_CACHE = {}


def _taylor_factors(bet_xy, bet_z, alpha):
    """Per-sample CP factors (A[9,R], U[15,R]) of the globally-normalized PSF."""
    zd = np.abs(np.arange(KZ) - KZ // 2).astype(np.float64)
    xd = np.abs(np.arange(KT) - KT // 2).astype(np.float64)
    dp = xd[:, None] ** 2 + xd[None, :] ** 2
    S = 0.0
    for b in range(B):
        bxy, bz, al = float(bet_xy[b]), float(bet_z[b]), float(alpha[b])
        lat = np.exp(-dp / (2 * bxy**2)) / (2 * np.pi * bxy**2)
        ax = np.exp(-zd**2 / (2 * bz**2)) / (np.sqrt(2 * np.pi) * bz)
        S += (1.0 - np.exp(-al * lat[None] * ax[:, None, None])).sum()
    facs = []
    for b in range(B):
        bxy, bz, al = float(bet_xy[b]), float(bet_z[b]), float(alpha[b])
        g = np.exp(-xd**2 / (2 * bxy**2))
        ax = np.exp(-zd**2 / (2 * bz**2)) / (np.sqrt(2 * np.pi) * bz)
        c = al * ax / (2 * np.pi * bxy**2)
        A = np.stack(
            [(-1) ** (m + 1) * c**m / math.factorial(m) / S for m in range(1, R + 1)], 1
        )
        U = np.stack([g**m for m in range(1, R + 1)], 1)
        facs.append((A.astype(np.float32), U.astype(np.float32)))
    return facs


def _build_mats(A, U):
    """Device weight matrices for one sample (built fp32, shipped bf16).

    ty0/ty1 hold the 4 ranks' y-Toeplitz blocks (96 outputs each) side by
    side so one matmul per (z, y-window) computes all ranks, and the two
    windows' psum blocks merge into a single copy.
    """
    ty0 = np.zeros((128, R * 96), np.float32)
    ty1 = np.zeros((128, R * 96), np.float32)
    tx = np.zeros((R, XIN, XH), np.float32)
    zm = np.zeros((128, Z), np.float32)
    for m in range(R):
        for yp in range(128):
            for yo in range(96):
                j = yp - yo          # window y_in [0:128] -> y_out 0..95
                if 0 <= j < KT:
                    ty0[yp, m * 96 + yo] = U[j, m]
                j = yp - yo - 18     # window y_in [78:206] -> y_out 96..191
                if 0 <= j < KT:
                    ty1[yp, m * 96 + yo] = U[j, m]
        for i in range(XIN):
            for o in range(max(0, i - KT + 1), min(XH, i + 1)):
                tx[m, i, o] = U[i - o, m]
        for zi in range(Z):
            for zo in range(max(0, zi - 4), min(Z, zi + 5)):
                zm[m * Z + zi, zo] = A[zi - zo + 4, m]
    return ty0, ty1, tx, zm


def _build_program(reps=1, upto=None):
    """upto: truncate after a stage ("A".."D") for stage-delta profiling;
    the last intermediate is DMA'd to out_d to keep it live through DCE."""
    import concourse.mybir as mybir
    import concourse.tile as tile
    from concourse import bacc

    F32, BF16 = mybir.dt.float32, mybir.dt.bfloat16

    nc = bacc.Bacc("TRN2", target_bir_lowering=False, debug=False, num_devices=NCORES)

    xin_d = nc.dram_tensor("xin", [Z, XIN, YIN], BF16, kind="ExternalInput")
    ty0_d = nc.dram_tensor("ty0", [128, R * 96], BF16, kind="ExternalInput")
    ty1_d = nc.dram_tensor("ty1", [128, R * 96], BF16, kind="ExternalInput")
    tx_d = nc.dram_tensor("tx", [R, XIN, XH], BF16, kind="ExternalInput")
    zm_d = nc.dram_tensor("zm", [128, Z], BF16, kind="ExternalInput")
    out_d = nc.dram_tensor("out", [Z, XH, Y], F32, kind="ExternalOutput")

    with tile.TileContext(nc) as tc:
        hwq = [nc.sync, nc.scalar]
        ncopy = [0]

        def copy(out, in_):
            # GPSIMD cannot read PSUM, so psum-evicting copies can only go
            # to Activation/DVE; let the scheduler balance those two.
            ncopy[0] += 1
            nc.any.tensor_copy(out=out, in_=in_)

        with (
            tc.tile_pool(name="consts", bufs=1) as consts,
            tc.tile_pool(name="persist", bufs=1) as persist,
            tc.tile_pool(name="psum", bufs=4, space="PSUM") as psp,
        ):
            # consts on the (otherwise idle) gpsimd queue, keeping the two
            # HWDGE rings free for the input transposes
            ty0 = consts.tile([128, R * 96], BF16)
            nc.gpsimd.dma_start(out=ty0[:], in_=ty0_d[:])
            ty1 = consts.tile([128, R * 96], BF16)
            nc.gpsimd.dma_start(out=ty1[:], in_=ty1_d[:])
            tx = [consts.tile([XIN, XH], BF16, name=f"tx_{m}") for m in range(R)]
            for m in range(R):
                nc.gpsimd.dma_start(out=tx[m][:], in_=tx_d[m])
            zmt = consts.tile([128, Z], BF16)
            nc.gpsimd.dma_start(out=zmt[:], in_=zm_d[:])

            # reps>1 (timing only): repeat the whole pipeline sequentially in
            # one NEFF; same-tag persistent tiles serialize the reps.
            for _rep in range(reps):
                # TT free layout: (t, g, z8, x110) -> block = t*4+g is one
                # contiguous 880-col chunk, written by one XBAR transpose.
                TT = persist.tile([128, 8 * 880], BF16, tag="tt")
                W = persist.tile([XIN, Z * R * Y], BF16, tag="w")
                # Xt/Wz split into per-y-chunk tiles so the SD/SE pipeline
                # gets exact tile-level dependencies.
                Xh = [
                    persist.tile([XH, 96 * R * Z], BF16, tag=f"xt{h}", name=f"xh{h}")
                    for h in range(2)
                ]
                Wzk = [
                    persist.tile([128, 48 * XH], BF16, tag=f"wz{k}", name=f"wzk{k}")
                    for k in range(4)
                ]
                Out = persist.tile([128, 24 * Y], F32, tag="outt")
                Wb = W[:].rearrange("p (z b q) -> p z b q", z=Z, b=2)
                Wc = W[:].rearrange("p (z b m y) -> p m z b y", z=Z, b=2, m=R)
                Xv = [
                    Xh[h][:].rearrange("p (y m z) -> p y m z", y=96, m=R, z=Z)
                    for h in range(2)
                ]
                Wzv = [Wzk[k][:].rearrange("p (y x) -> p y x", y=48) for k in range(4)]
                OutV = Out[:].rearrange("p (x y) -> p x y", x=24)

                # SA: 4 direct DRAM->SBUF transpose DMAs (2 y-windows x 2
                # z-groups of 16).  ALL transposes in this program go on the
                # sync ring: two concurrent XBAR transposes (any source)
                # corrupt each other on HW.  SB group g needs 2 blocks.
                for g in range(2):
                    for t in range(2):
                        src = xin_d[16 * g : 16 * (g + 1), :, 78 * t : 78 * t + 128]
                        nc.sync.dma_start_transpose(
                            TT[:, (t * 2 + g) * 1760 : (t * 2 + g + 1) * 1760],
                            src.rearrange("z x y -> (z x) y"),
                        )

                if upto == "A":
                    nc.gpsimd.dma_start(
                        out=out_d[:].rearrange("z x y -> z (x y)")[:, 0:7040],
                        in_=TT[0:Z, :],
                    )
                    continue

                # SB: y-conv, all ranks per z-plane in 2 matmuls + 1 copy
                def tts(zp, t):
                    s = (t * 2 + zp // 16) * 1760 + (zp % 16) * 110
                    return TT[:, s : s + 110]

                for zp in range(Z):
                    ps = psp.tile([128, 1024], F32, tag="ps")
                    nc.tensor.matmul(ps[:XIN, 0:384], tts(zp, 0), ty0[:])
                    nc.tensor.matmul(ps[:XIN, 512:896], tts(zp, 1), ty1[:])
                    for bb in range(2):
                        copy(
                            out=Wb[:, zp, bb],
                            in_=ps[:XIN, 512 * bb : 512 * bb + 384],
                        )

                if upto == "B":
                    nc.gpsimd.dma_start(
                        out=out_d[:].rearrange("z x y -> z (x y)"),
                        in_=W[0:Z, 0 : XH * Y],
                    )
                    continue

                # SC: x-conv, y-halved: per (h, m, z-octet) 2 z-quad matmuls
                # (N=384) + 1 copy.  Finishing half h unblocks SD chunks 2h(+1).
                for h in range(2):
                    for m in range(R):
                        for zq in range(4):
                            z0 = 8 * zq
                            ps = psp.tile([128, 1024], F32, tag="ps")
                            for i in range(2):
                                nc.tensor.matmul(
                                    ps[:XH, 512 * i : 512 * i + 384],
                                    tx[m][:],
                                    Wc[:, m, z0 + 4 * i : z0 + 4 * i + 4, h],
                                )
                            for bb in range(2):
                                copy(
                                    out=Xv[h][:, :, m, z0 + 4 * bb : z0 + 4 * bb + 4],
                                    in_=ps[:XH, 512 * bb : 512 * bb + 384].rearrange(
                                        "p (z y) -> p y z", z=4
                                    ),
                                )

                if upto == "C":
                    nc.gpsimd.dma_start(
                        out=out_d[:].rearrange("z x y -> z (x y)")[:, 0 : 96 * R * Z],
                        in_=Xh[0][0:Z, :],
                    )
                    continue

                # SD: (m,z)-transpose, 4 y-chunk tiles, serialized on the sync
                # ring (concurrent SBUF->SBUF XBAR transposes corrupt on HW).
                for k in range(4):
                    nc.sync.dma_start_transpose(
                        Wzv[k][:, :, :],
                        Xh[k // 2][:, (k % 2) * 48 * 128 : (k % 2 + 1) * 48 * 128],
                    )

                if upto == "D":
                    nc.gpsimd.dma_start(
                        out=out_d[:].rearrange("z x y -> z (x y)")[:, 0 : 48 * XH],
                        in_=Wzk[0][0:Z, :],
                    )
                    continue

                # SE: z-conv + rank sum per (y-chunk, x8-grain); consumes SD
                # chunk tiles as they land.
                for yb in range(4):
                    for xp in range(6):
                        ps = psp.tile([128, 1024], F32, tag="ps")
                        for i in range(2):
                            xo = 2 * xp + i
                            nc.tensor.matmul(
                                ps[:Z, 512 * i : 512 * i + 384],
                                zmt[:],
                                Wzv[yb][:, :, 8 * xo : 8 * (xo + 1)],
                            )
                            xg, xl = (8 * xo) // 24, (8 * xo) % 24
                            copy(
                                out=OutV[
                                    xg * Z : (xg + 1) * Z,
                                    xl : xl + 8,
                                    48 * yb : 48 * (yb + 1),
                                ],
                                in_=ps[:Z, 512 * i : 512 * i + 384].rearrange(
                                    "p (y x) -> p x y", y=48
                                ),
                            )

                # out DRAM [z, x, y] <- Out [(xg z), (xl y)]; one DMA per xg,
                # on the scalar ring so they never queue behind transposes.
                for xg in range(4):
                    nc.scalar.dma_start(
                        out=out_d[:, xg * 24 : (xg + 1) * 24, :],
                        in_=Out[xg * Z : (xg + 1) * Z, :],
                    )

    nc.compile()
    return nc


def _make_in_maps(x, bet_xy, bet_z, alpha):
    import ml_dtypes

    bf16 = ml_dtypes.bfloat16
    facs = _taylor_factors(np.asarray(bet_xy), np.asarray(bet_z), np.asarray(alpha))
    in_maps = []
    for c in range(NCORES):
        b, xh = c // 2, c % 2
        A, U = facs[b]
        ty0, ty1, tx, zm = _build_mats(A, U)
        xpad = np.zeros((Z, XIN, YIN), np.float32)
        x0 = XH * xh - 7
        lo, hi = max(0, x0), min(X, x0 + XIN)
        xpad[:, lo - x0 : hi - x0, 7 : 7 + Y] = x[b, 0, :, lo:hi, :]
        in_maps.append(
            {
                "xin": xpad.astype(bf16),
                "ty0": ty0.astype(bf16),
                "ty1": ty1.astype(bf16),
                "tx": tx.astype(bf16),
                "zm": zm.astype(bf16),
            }
        )
    return in_maps


def kernel(x, bet_xy, bet_z, alpha):
    from concourse.bass_utils import run_bass_kernel_spmd

    x = np.asarray(x, dtype=np.float32)

    if "nc" not in _CACHE:
        _CACHE["nc"] = _build_program()
    nc = _CACHE["nc"]

    in_maps = _make_in_maps(x, bet_xy, bet_z, alpha)
    res = run_bass_kernel_spmd(nc, in_maps, list(range(NCORES))).results

    out = np.empty((B, 1, Z, X, Y), np.float32)
    for c in range(NCORES):
        b, xh = c // 2, c % 2
        out[b, 0, :, XH * xh : XH * (xh + 1), :] = res[c]["out"]
    return out
